# revision 30
# baseline (speedup 1.0000x reference)
"""MoE (token-choice top-2 router + grouped SwiGLU experts + shared expert)
on 8 Trainium2 NeuronCores.

Sharding: expert-parallel — core e owns expert e's routed tokens (host
dispatch, capacity-padded), plus a 1/8 data-parallel slice of the shared
expert. Host does the (cheap) routing control plane: gate matmul, top-2
selection, stable sort by expert, gather/scale, and the final scatter-add
combine. The device kernel does all the FLOPs: per-core SwiGLU
  h = silu(x @ w1.T) * (x @ w3.T);  out = h @ w2.T
in bf16 with fp32 PSUM accumulation (matching the reference's bf16
grouped-mm semantics), for both the routed tokens and the shared slice.

Device-side structure (trace-driven, ~400us on 8 busy cores vs 485us for
the first working version):
- Out-projections are computed transposed (outT[D, tokens] = w2 @ g) so
  tokens live on the matmul free dim: no capacity padding to a multiple
  of 128, uniform free-dim tiles.
- Phase order: swiglu_r w1-pass, w3-pass, swiglu_s (both passes), out_r,
  out_s. The two swiglu_r passes are separate composable calls so that
  prefetch DMA issues can be emitted between them.
- Prefetch: xr / xs / sw2t are loaded into SBUF caches by DMAs issued on
  the scalar (Activation) queue — a hardware-DGE engine with fast
  descriptors (gpsimd.dma_start is the software-DGE path whose ~300ns
  descriptors clog all 16 DMA engines). Scalar-FIFO emission position
  paces them: xr first (first phase consumes it), xs+sw2t between the
  two swiglu_r passes (~95us in, when only the w3 weight stream competes
  for bandwidth). sw2t is fully cached in SBUF (44KB/partition) so the
  final out_s phase starts with zero DMA latency.
- swiglu_s uses K_TILE=1024 (2 weight-DMA issues per 3.5us m-tile
  instead of 4; the ~700-900ns DIRECT2D issue cost starves it otherwise).
- w2 (routed out-proj) streams in k-groups on the gpsimd queue, where
  its low rate tolerates slow software-DGE descriptors.

Self-contained: only needs numpy/ml_dtypes/concourse (the Bass stack).
"""

import math
import os

import numpy as np
import ml_dtypes

BF16 = ml_dtypes.bfloat16
NCORES = 8
TOP_K = 2
ROUTE_SCALE = 1.0

# filled by the last kernel() call (exec_time_ns etc. when tracing)
LAST = {}

_PROGRAM_CACHE = {}


def _install_profhook():
    """Best-effort shim for antenv.axon_hooks so trace=True can capture NTFF
    profiles in this container. Harmless no-op if anything is missing."""
    try:
        import sys
        import types

        if "antenv.axon_hooks" in sys.modules:
            return
        import trn_agent_boot.trn_boot as tb

        hook = tb._ntff_profile_via_ctypes("/opt/axon/libaxon_pjrt.so")
        m = types.ModuleType("antenv.axon_hooks")
        m._hook = hook
        m.set_axon_ntff_profile_hook = lambda h: setattr(m, "_hook", h)
        m.get_axon_ntff_profile_hook = lambda: m._hook
        import antenv

        sys.modules["antenv.axon_hooks"] = m
        antenv.axon_hooks = m

        import concourse.bass_utils as bu

        bu.upload_artifacts = lambda tmpdir: tmpdir
    except Exception:
        pass


def _free_div(n):
    """Largest f = n/k (k<=4) with f <= 512, preferring big f."""
    for k in (1, 2, 3, 4):
        if n % k == 0 and n // k <= 512:
            return n // k
    for f in (512, 384, 256, 128):
        if n % f == 0:
            return f
    raise ValueError(f"no free-dim divisor for {n}")


def _pick_ntok(nmax, cap):
    """Smallest n in [nmax, cap] whose free-dim divides nicely (PSUM <=512)."""
    for n in range(nmax, cap + 1):
        try:
            _free_div(n)
            return n
        except ValueError:
            continue
    return cap


def _build_program(D, H, NTOK, TS):
    import concourse.bacc as bacc
    import concourse.bass as bass
    import concourse.tile as tile
    from concourse import mybir
    from concourse.kernels.tile_matmul import (
        ShapeInfo,
        batched_producer_kxm,
        composable_matmul_tile_kernel,
        dma_from_dram_kxm,
    )
    from contextlib import ExitStack

    bf = mybir.dt.bfloat16
    f32 = mybir.dt.float32
    P = 128

    nc = bacc.Bacc(target_bir_lowering=False)

    xr = nc.dram_tensor("xr", [D, NTOK], bf, kind="ExternalInput")
    w1t = nc.dram_tensor("w1t", [D, H], bf, kind="ExternalInput")
    w3t = nc.dram_tensor("w3t", [D, H], bf, kind="ExternalInput")
    w2t = nc.dram_tensor("w2t", [H, D], bf, kind="ExternalInput")
    xs = nc.dram_tensor("xs", [D, TS], bf, kind="ExternalInput")
    sw1t = nc.dram_tensor("sw1t", [D, H], bf, kind="ExternalInput")
    sw3t = nc.dram_tensor("sw3t", [D, H], bf, kind="ExternalInput")
    sw2t = nc.dram_tensor("sw2t", [H, D], bf, kind="ExternalInput")
    # transposed outputs: tokens on the free dim
    outr = nc.dram_tensor("outr", [D, NTOK], bf, kind="ExternalOutput")
    outs = nc.dram_tensor("outs", [D, TS], f32, kind="ExternalOutput")

    FREE_R = _free_div(NTOK)  # e.g. 361 for NTOK=1083
    FREE_S = _free_div(TS)  # 512

    with tile.TileContext(nc) as tc, ExitStack() as ctx:
        caches = ctx.enter_context(tc.tile_pool(name="caches", bufs=1))
        xcache = caches.tile([P, D // P, NTOK], bf, tag="xcache")
        xscache = caches.tile([P, D // P, TS], bf, tag="xscache")
        h1cache = caches.tile([P, H // P, NTOK], bf, tag="h1cache")
        h1scache = caches.tile([P, H // P, TS], bf, tag="h1scache")
        swcache = caches.tile([P, H // P, D], bf, tag="swcache")

        # ---- input prefetch ----
        # All prefetches go through the scalar (Activation) queue: it is a
        # hardware-DGE engine (fast descriptors). gpsimd.dma_start is the
        # software-DGE path whose ~300ns/descriptor grind clogs all 16 DMA
        # engines (v4 trace). Per-po [128,1,*] transfers keep descriptors
        # clean. Ordering within the scalar FIFO does the pacing: xr is
        # issued before swiglu_r; xs/sw2t are emitted between swiglu_r's two
        # weight passes, so they enter the queues around t~95us when only
        # the w3 weight stream competes for bandwidth.
        xr3 = xr[:].rearrange("(po pi) f -> pi po f", pi=P)
        xs3 = xs[:].rearrange("(po pi) f -> pi po f", pi=P)
        sw2t3 = sw2t[:].rearrange("(po pi) f -> pi po f", pi=P)
        KSUB = 512 // P
        # first k-tile split per-po so the very first matmul unblocks early
        for po in range(KSUB):
            nc.scalar.dma_start(
                out=xcache[:, po : po + 1, :], in_=xr3[:, po : po + 1, :]
            )
        for t in range(1, D // 512):
            nc.scalar.dma_start(
                out=xcache[:, t * KSUB : (t + 1) * KSUB, :],
                in_=xr3[:, t * KSUB : (t + 1) * KSUB, :],
            )

        # PE warm-up: the HAM clock gate keeps the array at 1.2GHz until
        # ~3.4us of sustained activity. Burn dummy matmuls on scratch SBUF
        # during the otherwise-idle window while the first xr/weight tiles
        # stream in, so the real matmuls start at full clock.
        with tc.tile_pool(name="warm", bufs=1) as wpool, tc.tile_pool(
            name="warmp", bufs=1, space="PSUM"
        ) as wpsum:
            wlhs = wpool.tile([P, 128], bf, tag="wlhs")
            wrhs = wpool.tile([P, 512], bf, tag="wrhs")
            nc.vector.memset(wlhs[:], 0.0)
            nc.vector.memset(wrhs[:], 0.0)
            wps = wpsum.tile([P, 512], f32)
            for _ in range(9):
                nc.tensor.matmul(wps[:], wlhs[:], wrhs[:], start=True, stop=True)

        def swiglu_batch(label, wap, xc, h1c, outc, M_COLS, FREE, mul_with=None,
                         KTS=512, WBUFS=7, pool=None):
            """One pass: outc = silu(w @ x) (mul_with=None) or
            outc = (w @ x) * mul_with."""
            kxm_pool = pool or ctx.enter_context(
                tc.tile_pool(name=f"wp_{label}", bufs=WBUFS)
            )
            kxm_producer, kxm_shape = dma_from_dram_kxm(kxm_pool, wap[:])

            def kxn_producer(nc_, md):
                return xc[
                    :,
                    bass.ts(md.k_tile_idx, md.k_subtiles),
                    bass.ts(md.n_tile_idx, md.n_tile),
                ]

            kxn_shape = ShapeInfo(pdims=((P, D // P),), fdims=(M_COLS,))

            def producer(nc_, md):
                return outc[
                    :,
                    bass.ts(md.m_tile_idx, md.m_subtiles),
                    bass.ts(md.n_tile_idx, md.n_tile),
                ]

            def reducer(nc_, psum, sbuf, md):
                if mul_with is None:
                    nc_.scalar.activation(
                        sbuf, psum, mybir.ActivationFunctionType.Silu
                    )
                else:
                    start = md.n_tile_idx * md.n_tile + md.n_subtile_idx * md.n_subtile
                    sz = md.n_subtile_slice_size
                    po = md.m_tile_idx * md.m_subtiles + md.m_subtile_idx
                    nc_.vector.tensor_mul(
                        out=sbuf,
                        in0=psum[:, :sz],
                        in1=mul_with[:, po, start : start + sz],
                    )

            composable_matmul_tile_kernel(
                tc=tc,
                kxm_shape=kxm_shape,
                kxn_shape=kxn_shape,
                output_type=bf,
                kxm_producer=kxm_producer,
                kxn_producer=kxn_producer,
                mxn_subtile_producer=producer,
                mxn_subtile_reducer=reducer,
                mxn_consumer=lambda nc_, sbuf, md: None,
                MATMUL_FREE_DIM=FREE,
                MAX_TILE_SIZE=max(M_COLS, 128),
                MAX_K_TILE_SIZE=KTS,
                psum_n_bufs=2,
            )
            return kxm_pool

        def out_proj_t(label, gc, out_ap, out_dt, M_COLS, FREE, w2ap=None, wcache=None):
            """outT[D, M_COLS] = w2 @ g.  kxm = w2T[H, D] (stream from DRAM or
            slice from SBUF cache), kxn = g[H, M_COLS] in SBUF."""
            KT = H // P  # 11 k-tiles of 128

            if wcache is not None:

                def pm(nc_, md):
                    return wcache[
                        :,
                        bass.ts(md.k_tile_idx, md.k_subtiles),
                        bass.ts(md.m_tile_idx, md.m_tile),
                    ]
            else:
                # W2 strips in k-groups of <=4, issued from GpSimd
                GRP = 4
                w2pool = ctx.enter_context(
                    tc.tile_pool(name=f"w2p_{label}", bufs=2)
                )
                w2_3d = w2ap[:].rearrange("(po pi) f -> pi po f", pi=P)
                state = {"m": None, "grp": {}}

                def pm(nc_, md):
                    if state["m"] != md.m_tile_idx:
                        state["m"] = md.m_tile_idx
                        state["grp"] = {}
                        cols = bass.ts(md.m_tile_idx, md.m_tile)
                        for g0 in range(0, KT, GRP):
                            g1 = min(g0 + GRP, KT)
                            t = w2pool.tile(
                                [P, g1 - g0, md.m_tile], bf, tag=f"w2g_{label}_{g0}"
                            )
                            nc_.gpsimd.dma_start(out=t[:], in_=w2_3d[:, g0:g1, cols])
                            for k in range(g0, g1):
                                state["grp"][k] = t[:, k - g0 : k - g0 + 1, :]
                    return state["grp"][md.k_tile_idx]

            sm = ShapeInfo(pdims=((P, KT),), fdims=(D,))

            def pn(nc_, md):
                return gc[
                    :,
                    bass.ts(md.k_tile_idx, md.k_subtiles),
                    bass.ts(md.n_tile_idx, md.n_tile),
                ]

            sn = ShapeInfo(pdims=((P, KT),), fdims=(M_COLS,))

            out3 = out_ap[:].rearrange("(po pi) f -> pi po f", pi=P)
            eng = "scalar" if label == "r" else "sync"

            def consumer(nc_, mxn_tile, md):
                n_sz = min(md.n_tile, M_COLS - md.n_tile_idx * md.n_tile)
                e = getattr(nc_, eng)
                if wcache is not None:
                    # last phase: per-po DMAs so the final flush is small
                    for po in range(md.m_subtiles):
                        e.dma_start(
                            out=out3[
                                :,
                                md.m_tile_idx * md.m_subtiles + po,
                                bass.ds(md.n_tile_idx * md.n_tile, n_sz),
                            ],
                            in_=mxn_tile[:, po, :n_sz],
                        )
                else:
                    e.dma_start(
                        out=out3[
                            :,
                            bass.ts(md.m_tile_idx, md.m_subtiles),
                            bass.ds(md.n_tile_idx * md.n_tile, n_sz),
                        ],
                        in_=mxn_tile[:, :, :n_sz],
                    )

            composable_matmul_tile_kernel(
                tc=tc,
                kxm_shape=sm,
                kxn_shape=sn,
                output_type=out_dt,
                kxm_producer=pm,
                kxn_producer=pn,
                mxn_consumer=consumer,
                MATMUL_FREE_DIM=FREE,
                MAX_TILE_SIZE=FREE,
                MAX_K_TILE_SIZE=512,
                temps_n_bufs=1 if wcache is not None else 2,
                psum_n_bufs=2,
            )

        # r first: its big m-tiles tolerate the cold start; swiglu_s (small
        # 3.8us m-tiles, marginal weight-issue rate) runs mid-kernel, warm,
        # with K_TILE=1024 halving its DMA issue count.
        rpool = swiglu_batch("r1", w1t, xcache, h1cache, h1cache, NTOK, FREE_R)
        # xs + sw2t prefetch issues land here in the scalar FIFO: after the
        # w1-pass silus (~95us), long before their consumers (~200us/~370us)
        for po in range(D // P):
            nc.scalar.dma_start(
                out=xscache[:, po : po + 1, :], in_=xs3[:, po : po + 1, :]
            )
        for po in range(H // P):
            nc.scalar.dma_start(
                out=swcache[:, po : po + 1, :], in_=sw2t3[:, po : po + 1, :]
            )
        swiglu_batch("r3", w3t, xcache, h1cache, h1cache, NTOK, FREE_R,
                     mul_with=h1cache, pool=rpool)
        spool = swiglu_batch("s1", sw1t, xscache, h1scache, h1scache, TS, FREE_S,
                             KTS=1024, WBUFS=4)
        swiglu_batch("s3", sw3t, xscache, h1scache, h1scache, TS, FREE_S,
                     mul_with=h1scache, KTS=1024, pool=spool)
        out_proj_t("r", h1cache, outr, bf, NTOK, FREE_R, w2ap=w2t)
        out_proj_t("s", h1scache, outs, f32, TS, FREE_S, wcache=swcache)

    nc.compile()
    return nc


def _route(x, gate_w, expert_bias):
    """Host control plane mirroring the reference routing exactly."""
    BS, SLEN, D = x.shape
    T = BS * SLEN
    xt = np.ascontiguousarray(x.reshape(T, D), dtype=np.float32)
    logits = xt @ gate_w.astype(np.float32).T  # [T, E]
    scores = 1.0 / (1.0 + np.exp(-logits))
    biased = scores + np.asarray(expert_bias, np.float32)[None, :]
    sel = np.argsort(-biased, axis=1, kind="stable")[:, :TOP_K]  # [T, K]
    top_scores = np.take_along_axis(scores, sel, axis=1) * ROUTE_SCALE
    sel_flat = sel.reshape(-1)
    order = np.argsort(sel_flat, kind="stable")  # [T*K]
    counts = np.bincount(sel_flat, minlength=NCORES)
    tok_idx = order // TOP_K
    scores_sorted = top_scores.reshape(-1)[order].astype(np.float32)
    return xt, counts, tok_idx, scores_sorted


def kernel(x, gate_w, w1, w2, w3, sw1, sw2, sw3, expert_bias):
    from concourse.bass_utils import run_bass_kernel_spmd

    x = np.asarray(x, np.float32)
    gate_w = np.asarray(gate_w, np.float32)
    w1 = np.asarray(w1, np.float32)
    w2 = np.asarray(w2, np.float32)
    w3 = np.asarray(w3, np.float32)
    sw1 = np.asarray(sw1, np.float32)
    sw2 = np.asarray(sw2, np.float32)
    sw3 = np.asarray(sw3, np.float32)
    expert_bias = np.asarray(expert_bias, np.float32)
    BS, SLEN, D = x.shape
    T = BS * SLEN
    H = w1.shape[1]
    TS = T // NCORES

    xt, counts, tok_idx, scores_sorted = _route(x, gate_w, expert_bias)
    off = np.concatenate([[0], np.cumsum(counts)]).astype(np.int64)
    CAP = max(128, int(math.ceil(counts.max() / 128) * 128))
    NTOK = _pick_ntok(max(128, int(counts.max())), CAP)

    key = (D, H, NTOK, TS)
    if key not in _PROGRAM_CACHE:
        _PROGRAM_CACHE[key] = _build_program(D, H, NTOK, TS)
    nc = _PROGRAM_CACHE[key]

    # stage per-core inputs
    sw1t_h = np.ascontiguousarray(np.asarray(sw1, np.float32).T).astype(BF16)
    sw3t_h = np.ascontiguousarray(np.asarray(sw3, np.float32).T).astype(BF16)
    sw2t_h = np.ascontiguousarray(np.asarray(sw2, np.float32).T).astype(BF16)
    in_maps = []
    for e in range(NCORES):
        n_e = int(counts[e])
        idx = tok_idx[off[e] : off[e] + n_e]
        seg = xt[idx] * scores_sorted[off[e] : off[e] + n_e, None]  # [n_e, D] f32
        xrT = np.zeros((D, NTOK), BF16)
        xrT[:, :n_e] = seg.T.astype(BF16)
        in_maps.append(
            {
                "xr": xrT,
                "w1t": np.ascontiguousarray(np.asarray(w1[e], np.float32).T).astype(BF16),
                "w3t": np.ascontiguousarray(np.asarray(w3[e], np.float32).T).astype(BF16),
                "w2t": np.ascontiguousarray(np.asarray(w2[e], np.float32).T).astype(BF16),
                "xs": np.ascontiguousarray(xt[e * TS : (e + 1) * TS].T).astype(BF16),
                "sw1t": sw1t_h,
                "sw3t": sw3t_h,
                "sw2t": sw2t_h,
            }
        )

    trace = os.environ.get("KERNEL_TRACE", "") not in ("", "0")
    if trace:
        _install_profhook()
    res = run_bass_kernel_spmd(
        nc, in_maps, list(range(NCORES)), trace=trace
    )
    LAST["exec_time_ns"] = res.exec_time_ns
    LAST["results"] = res

    # combine: shared slices + per-expert scatter-add (outputs are [D, tokens])
    out = np.empty((T, D), np.float32)
    for c in range(NCORES):
        out[c * TS : (c + 1) * TS] = res.results[c]["outs"].T
    for e in range(NCORES):
        n_e = int(counts[e])
        if n_e:
            idx = tok_idx[off[e] : off[e] + n_e]
            out[idx] += res.results[e]["outr"][:, :n_e].T.astype(np.float32)
    return out.reshape(BS, SLEN, D)
